# revision 16
# baseline (speedup 1.0000x reference)
"""BAD-descriptor kernel for Trainium2 (8 NeuronCores) — fp8 DoubleRow
TensorEngine window-sum formulation.

Math: the reference's integral-image box difference
    out[p, y, x] = S1/area - S2/area - th
is computed over precomputed d-wide horizontal window sums of the padded
image (d = 2r+1 in {3,5,7}):
    box(y, x; oy, ox, r) = sum_{dr in [-r,r]} K_d[y+oy+20+dr, (19+ox-r)+x]
K_d is stored as an fp8e4m3 hi/lo pair (K ~= K_hi + K_lo, ~1e-3 relative
error after the box difference) laid out as [row, chunk, {hi,lo}, col].

Per (pair, chunk, PSUM bank): each box is ONE fp8 MatmulPerfMode.DoubleRow
matmul whose two k-tiles are the hi and lo planes (adjacent blocks at even
stride 680, as the ISA requires); the banded +-1 weights are duplicated
into both k-tile slots, so one matmul accumulates band.(hi+lo) at
0.5 cycles/output column.  Two boxes accumulate into the same PSUM region
(start/stop flags) -> 1 cycle/output column total, 2x faster than fp16.

Row phases: the oy offset in [-16,16] is split as oy+16 = 16*phi + rho
(phi in {0,1}).  Each (d, phi) K tensor re-chunks the rows at offset
16*phi, so the weight band only spans rho+2r+1 <= 23 rows and a chunk of
CHUNK=96 output rows fits the 128-partition contraction (XROWS=119,
zero-padded to 128 — the HW requires full 128-partition DoubleRow
matmuls with dst partition base 0).

480 rows = 5 uniform chunks of 96.  Per chunk, evacuation reads
[96, 640] PSUM and applies scale=1/area, bias=-th while converting to
fp16.  Chunks {0,2,4} go to tile otX, {1,3} to otY; ACT takes group X on
3 of 4 pairs and group Y otherwise (DVE the complement), balancing
ACT:DVE evacuation columns 88:72 per core.  Two output DMAs per pair
(fp16, 3.8 KB descriptors) keep the SP sequencer's ~0.7 us per-DMA issue
cost off the critical path.

Sharding: 32 pairs per core, one SPMD program with 8 partition-id
branches (weights are per-core inputs, column offsets compile-time).
Clamped edge strips are recomputed on host (<~5% of elements).
Host converts the fp16 [pair, 96, group-chunk, col] device layout to the
full fp32 [1, 256, 480, 640] output.

The program takes a runtime `reps` scalar (uint32) repeating the whole
computation on-device; kernel() passes 1.  test.py uses large reps to
measure per-iteration HW time free of host/transfer noise.
"""

import numpy as np
import ml_dtypes

H, W = 480, 640
MR = 3
P_TOTAL = 256
N_CORES = 8
PAIRS_PER_CORE = P_TOTAL // N_CORES
CHUNK = 96
NCHUNK = H // CHUNK          # 5
KROWS = 519                  # rows of the padded window-sum image
XROWS = 119                  # rows used per chunk (band max 118), pad to 128
KP = 128                     # contraction partitions (HW requirement)
XCOLS = 679                  # K col j <-> window start xp col j-16
KC = 680                     # stored col count (hi->lo stride, even)
PS_W = 512                   # PSUM bank width in fp32
GX = (0, 2, 4)               # chunk group X -> otX
GY = (1, 3)                  # chunk group Y -> otY
DS = (3, 5, 7)
F8 = ml_dtypes.float8_e4m3


def _integral(xs: np.ndarray) -> np.ndarray:
    """(487, 647) float32 integral image (for the host edge fixup)."""
    xp = np.pad(xs, MR, mode="edge")
    ii = np.zeros((H + 2 * MR + 1, W + 2 * MR + 1), dtype=np.float32)
    np.cumsum(np.cumsum(xp, axis=0, dtype=np.float32), axis=1,
              dtype=np.float32, out=ii[1:, 1:])
    return ii


def _box_geom(oy, ox, r):
    """(phi, rho, xb) for one box: oy+16 = 16*phi + rho, K col = xb + x."""
    t = int(oy) + 16
    phi = 1 if t >= 16 else 0
    rho = t - 16 * phi          # in [0, 16]
    xb = 19 + int(ox) - r
    return phi, rho, xb


def _make_host_inputs(x, off_x1, off_y1, off_x2, off_y2, radii):
    """K hi/lo phase tensors [(d,phi) -> (128, 5, 2, 680) fp8] and per-core
    banded +-1 weights [KP, PAIRS_PER_CORE, 2 boxes, 2 ktiles, CHUNK]."""
    xs = np.asarray(x[0, 0], dtype=np.float32)
    xp = np.pad(xs, MR, mode="edge")                 # (486, 646)
    xe = np.pad(xp, ((17, 16), (16, 29)), mode="edge").astype(np.float64)
    ce = np.cumsum(xe, axis=1)
    ce = np.concatenate([np.zeros((ce.shape[0], 1)), ce], axis=1)
    kd = {}
    for d in DS:
        K = ce[:, d:d + XCOLS] - ce[:, 0:XCOLS]      # (519, 679) f64
        hilo = np.zeros((KROWS, 2, KC), F8)
        hi = K.astype(F8)
        hilo[:, 0, :XCOLS] = hi
        hilo[:, 1, :XCOLS] = (K - hi.astype(np.float64)).astype(F8)
        for phi in (0, 1):
            t = np.zeros((KP, NCHUNK, 2, KC), F8)
            for c in range(NCHUNK):
                r0 = CHUNK * c + 16 * phi
                t[:XROWS, c] = hilo[r0:r0 + XROWS]
            kd[f"k{d}p{phi}"] = np.ascontiguousarray(t)

    wts = []
    j = np.arange(CHUNK)
    for c in range(N_CORES):
        w = np.zeros((KP, PAIRS_PER_CORE, 2, 2, CHUNK), np.float32)
        for k in range(PAIRS_PER_CORE):
            p = c * PAIRS_PER_CORE + k
            r = int(radii[p])
            for b, (oy, ox) in enumerate(((off_y1[p], off_x1[p]),
                                          (off_y2[p], off_x2[p]))):
                _, rho, _ = _box_geom(oy, ox, r)
                sgn = 1.0 if b == 0 else -1.0
                for dr in range(-r, r + 1):
                    w[j + rho + 4 + dr, k, b, 0, j] += sgn
            w[:, k, :, 1, :] = w[:, k, :, 0, :]      # duplicate into ktile 1
        wts.append(np.ascontiguousarray(w.astype(F8)))
    return kd, wts, _integral(xs)


def _build_program(off_y1, off_x1, off_y2, off_x2, radii, thresholds,
                   timing_consts=None):
    """timing_consts=None: production program (K tensors are inputs, the
    full result is an ExternalOutput).  timing_consts=kd dict: timing
    variant — K baked into the NEFF, result device-local, so one run's
    host<->device traffic is the per-core weights + a scalar.  The
    per-rep instruction stream is identical either way."""
    import concourse.tile as tile
    from concourse import bacc, mybir
    from concourse.bass import MemorySpace
    import contextlib

    DT = mybir.dt.float32
    F16 = mybir.dt.float16
    FP8 = mybir.dt.float8e4
    DR = mybir.MatmulPerfMode.DoubleRow
    knames = [f"k{d}p{phi}" for d in DS for phi in (0, 1)]
    nc = bacc.Bacc()
    if timing_consts is None:
        k_ext = {n: nc.declare_dram_parameter(
            n, [KP, NCHUNK, 2, KC], FP8, isOutput=False) for n in knames}
    else:
        k_ext = {n: nc.inline_tensor(timing_consts[n], name=f"{n}c")
                 for n in knames}
    wts_ext = nc.declare_dram_parameter(
        "wts", [KP, PAIRS_PER_CORE, 2, 2, CHUNK], FP8, isOutput=False)
    reps_ext = nc.declare_dram_parameter("reps", [1, 1], mybir.dt.uint32,
                                         isOutput=False)
    if timing_consts is None:
        outx_ext = nc.declare_dram_parameter(
            "outx", [PAIRS_PER_CORE, CHUNK, len(GX), W], F16, isOutput=True)
        outy_ext = nc.declare_dram_parameter(
            "outy", [PAIRS_PER_CORE, CHUNK, len(GY), W], F16, isOutput=True)
    else:
        outx_ext = nc.dram_tensor("outx_i",
                                  [PAIRS_PER_CORE, CHUNK, len(GX), W],
                                  F16, kind="Internal")
        outy_ext = nc.dram_tensor("outy_i",
                                  [PAIRS_PER_CORE, CHUNK, len(GY), W],
                                  F16, kind="Internal")
        done_ext = nc.declare_dram_parameter("done", [1, 1], DT,
                                             isOutput=True)

    with tile.TileContext(nc) as tc:
        with contextlib.ExitStack() as ctx:
            ipool = ctx.enter_context(tc.tile_pool(name="ipool", bufs=1))
            opool = ctx.enter_context(tc.tile_pool(name="opool", bufs=6))
            pspool = ctx.enter_context(
                tc.tile_pool(name="pspool", bufs=2, space=MemorySpace.PSUM))

            kt = {}
            for n in knames:
                kt[n] = ipool.tile([KP, NCHUNK, 2, KC], FP8, tag=n, name=n)
                nc.sync.dma_start(kt[n][:], k_ext[n][:])
            wtt = ipool.tile([KP, PAIRS_PER_CORE, 2, 2, CHUNK], FP8)
            nc.sync.dma_start(wtt[:], wts_ext[:])

            def pe_pair(c, k):
                p = c * PAIRS_PER_CORE + k
                r = int(radii[p]); d = 2 * r + 1
                inv_area = 1.0 / float(d * d)
                th = float(thresholds[p])
                boxes = []
                for b, (oy, ox) in enumerate(((off_y1[p], off_x1[p]),
                                              (off_y2[p], off_x2[p]))):
                    phi, _, xb = _box_geom(oy, ox, r)
                    boxes.append((kt[f"k{d}p{phi}"], xb))
                # ACT takes group X (chunks 0,2,4) on 19 of 32 pairs
                # -> ACT:DVE = 83:77 chunks, equalizing engine busy
                act_x = ((k * 19) % 32) < 19
                otX = opool.tile([CHUNK, len(GX), W], F16, tag="otX")
                otY = opool.tile([CHUNK, len(GY), W], F16, tag="otY")
                # box-outer over 2-chunk groups: runs of 4 matmuls share one
                # stationary (fewer PE weight switches)
                for grp in ((0, 1), (2, 3), (4,)):
                    pst = {}
                    for cch in grp:
                        use_act = ((cch in GX) == act_x)
                        tag = "psA" if use_act else "psB"
                        pst[cch] = pspool.tile([KP, 2 * PS_W], DT, tag=tag,
                                               name=tag)
                    for b, (kb, xb) in enumerate(boxes):
                        lhsT = wtt[:, k, b, :, :]
                        for cch in grp:
                            for (s, e) in ((0, PS_W), (PS_W, W)):
                                nc.tensor.matmul(
                                    pst[cch][0:CHUNK, s:e], lhsT,
                                    kb[:, cch, :, xb + s:xb + e],
                                    start=(b == 0), stop=(b == 1),
                                    perf_mode=DR)
                    for cch in grp:
                        in_x = cch in GX
                        use_act = (in_x == act_x)
                        ot = otX if in_x else otY
                        dst = ot[:, (GX if in_x else GY).index(cch), :]
                        if use_act:
                            nc.scalar.activation(
                                dst, pst[cch][0:CHUNK, 0:W],
                                mybir.ActivationFunctionType.Copy,
                                bias=-th, scale=inv_area)
                        else:
                            nc.vector.tensor_scalar(
                                dst, pst[cch][0:CHUNK, 0:W],
                                inv_area, -th,
                                mybir.AluOpType.mult, mybir.AluOpType.add)
                nc.sync.dma_start(outx_ext[k], otX[:])
                nc.sync.dma_start(outy_ext[k], otY[:])

            tmp = nc.alloc_registers("reps_regs", mybir.ALL_ENGINES)
            nc.regs_load(tmp, reps_ext[0:1, 0:1])
            rv = nc.snap(tmp, donate=True, min_val=0, max_val=1 << 20)

            pid = nc.partition_id()
            for c in range(N_CORES):
                with tc.If(pid == c):
                    with tc.For_i(0, rv):
                        for k in range(PAIRS_PER_CORE):
                            pe_pair(c, k)
                        if timing_consts is not None:
                            dn = opool.tile([1, 1], DT, tag="dn", name="dn")
                            nc.vector.tensor_copy(
                                dn[:], kt[knames[0]][0:1, 0:1, 0:1, 0:1])
                            nc.sync.dma_start(done_ext[:], dn[:])
    nc.finalize()
    return nc


def _host_edges(out, I2D, off_y1, off_x1, off_y2, off_x2, radii, thresholds):
    """Recompute (on host, mirroring the reference exactly) every output
    element whose box center got clamped."""
    ally = np.arange(H, dtype=np.float32)
    allx = np.arange(W, dtype=np.float32)

    def box(oy, ox, r, ys, xs):
        cy = (np.clip(ys + oy, 0.0, float(H - 1))).astype(np.int32) + MR
        cx = (np.clip(xs + ox, 0.0, float(W - 1))).astype(np.int32) + MR
        y0 = (cy - r)[:, None]; y1 = (cy + r + 1)[:, None]
        x0 = (cx - r)[None, :]; x1 = (cx + r + 1)[None, :]
        area_sum = (I2D[y1, x1] - I2D[y0, x1] - I2D[y1, x0] + I2D[y0, x0])
        return area_sum / np.float32((2 * r + 1) ** 2)

    for p in range(P_TOTAL):
        oy1 = float(off_y1[p]); ox1 = float(off_x1[p])
        oy2 = float(off_y2[p]); ox2 = float(off_x2[p])
        r = int(radii[p]); th = np.float32(thresholds[p])
        t = int(max(0.0, -oy1, -oy2)); b = int(max(0.0, oy1, oy2))
        l = int(max(0.0, -ox1, -ox2)); rr = int(max(0.0, ox1, ox2))

        def patch(ys, xs):
            out[p, ys[:, None].astype(np.int32), xs[None, :].astype(np.int32)] = (
                box(oy1, ox1, r, ys, xs) - box(oy2, ox2, r, ys, xs) - th)

        if t:
            patch(ally[:t], allx)
        if b:
            patch(ally[H - b:], allx)
        if l:
            patch(ally, allx[:l])
        if rr:
            patch(ally, allx[W - rr:])
    return out


def _assemble(outx, outy):
    """[PPC, 96, 3, W] + [PPC, 96, 2, W] fp16 -> [PPC, 480, W] fp32."""
    n = outx.shape[0]
    full = np.empty((n, H, W), np.float32)
    for i, cch in enumerate(GX):
        full[:, CHUNK * cch:CHUNK * (cch + 1)] = \
            outx[:, :, i].astype(np.float32)
    for i, cch in enumerate(GY):
        full[:, CHUNK * cch:CHUNK * (cch + 1)] = \
            outy[:, :, i].astype(np.float32)
    return full


def _run(x, offset_x1, offset_x2, offset_y1, offset_y2, radii, thresholds):
    from concourse.bass_utils import run_bass_kernel_spmd

    x = np.asarray(x); radii_np = np.asarray(radii)
    off_x1 = np.asarray(offset_x1); off_x2 = np.asarray(offset_x2)
    off_y1 = np.asarray(offset_y1); off_y2 = np.asarray(offset_y2)
    th_np = np.asarray(thresholds)

    kd, wts, I2D = _make_host_inputs(x, off_x1, off_y1, off_x2, off_y2,
                                     radii_np)
    nc = _build_program(off_y1, off_x1, off_y2, off_x2, radii_np, th_np)
    in_maps = [{**kd, "wts": wts[c], "reps": np.array([[1]], np.uint32)}
               for c in range(N_CORES)]
    bkr = run_bass_kernel_spmd(nc, in_maps, list(range(N_CORES)))

    out = np.concatenate(
        [_assemble(np.asarray(bkr.results[c]["outx"]),
                   np.asarray(bkr.results[c]["outy"]))
         for c in range(N_CORES)], axis=0)
    out = _host_edges(out, I2D, off_y1, off_x1, off_y2, off_x2, radii_np,
                      th_np)
    return out[None].astype(np.float32, copy=False)


def kernel(x, offset_x1, offset_x2, offset_y1, offset_y2, radii, thresholds):
    return _run(x, offset_x1, offset_x2, offset_y1, offset_y2, radii,
                thresholds)
